# revision 16
# baseline (speedup 1.0000x reference)
"""Trainium2 Bass kernel for nn_GCNModelFeaAttScatStructureOnlyVAE.

Computation (reference):
    h_att  = relu(x @ W_att1.T)                    [N, D1]
    score  = softmax(h_att @ W_att2.T, axis=1)     [N, D2]
    cond   = x * score                             [N, D2]
    support= cond @ W_gcn                          [N, D1]
    h      = relu(adj @ support)                   [N, D1]
    z      = batchnorm(h) (batch stats, biased var) * gamma + beta
    out    = z @ z.T                               [N, N]

Distribution (8 NeuronCores, row-sharded nodes):
  Each core owns R = N/8 rows of x/adj/out. Stage A (attention+support) is
  computed per-core on its row block; support is AllGathered (bf16). The
  big matmul adj_blk @ support runs in TRANSPOSED orientation
  (h^T[f, r] = sum_c support[c, f] * adjT[c, r]) so BatchNorm becomes a
  free-dim reduction and z^T comes out pre-transposed for the z @ z.T
  decode. adj tiles are cast to bf16 during the DMA load (SWDGE) and
  transposed on-chip by the PE. BN stats are reduced inside the matmul-1
  epilogue and AllReduced (4 KB); z^T is AllGathered in two node-halves
  (bf16) so the second half's transfer overlaps the first half's matmuls.

  Matmul operands are bf16 (full-rate PE + fast weight load); all
  accumulation is fp32 in PSUM; softmax/BN statistics are fp32.
"""
import os
import sys

for _p in ("/opt/trn_rl_repo", "/root/.axon_site/_ro/trn_rl_repo"):
    if _p not in sys.path:
        sys.path.append(_p)

import numpy as np

import concourse.bass as bass
import concourse.mybir as mybir
import concourse.tile as tile
from concourse import bacc, bass_utils
from concourse.masks import make_identity

f32 = mybir.dt.float32
f32r = mybir.dt.float32r
bf16 = mybir.dt.float16  # fp16: same PE rate as bf16, 4x finer mantissa
AX = mybir.AxisListType
ALU = mybir.AluOpType
ACT = mybir.ActivationFunctionType

N = 8192
D2 = 128
D1 = 512
NCORES = 8
R = N // NCORES          # 1024 rows per core
BN_EPS = 1e-5
P = 128

RT = R // P              # 8 row tiles per core
FC = D1 // P             # 4 feature chunks
RS = 2                   # row-slices per core in mm1 (512 rows each)
RSW = R // RS            # 512
CB = 16                  # c-superchunks (512 cols each)
CBW = N // CB            # 512


def build_kernel():
    nc = bacc.Bacc("TRN2", target_bir_lowering=False, debug=False,
                   num_devices=NCORES)

    x_d = nc.dram_tensor("x_blk", [R, D2], f32, kind="ExternalInput").ap()
    adj_d = nc.dram_tensor("adj_blk", [R, N], f32, kind="ExternalInput").ap()
    w1_d = nc.dram_tensor("W_att1", [D1, D2], f32, kind="ExternalInput").ap()
    w2_d = nc.dram_tensor("W_att2", [D2, D1], f32, kind="ExternalInput").ap()
    wg_d = nc.dram_tensor("W_gcn", [D2, D1], f32, kind="ExternalInput").ap()
    gamma_d = nc.dram_tensor("gamma", [D1], f32, kind="ExternalInput").ap()
    beta_d = nc.dram_tensor("beta", [D1], f32, kind="ExternalInput").ap()
    out_d = nc.dram_tensor("out_blk", [R, N], f32, kind="ExternalOutput").ap()

    rg = [list(range(NCORES))]

    with tile.TileContext(nc) as tc:
        with tc.tile_pool(name="const", bufs=1) as const, \
             tc.tile_pool(name="stagea", bufs=1) as sa, \
             tc.tile_pool(name="work", bufs=3) as work, \
             tc.tile_pool(name="precast", bufs=2) as precast, \
             tc.tile_pool(name="adjtp", bufs=6) as adjtp, \
             tc.tile_pool(name="rhsp", bufs=2) as rhsp, \
             tc.tile_pool(name="outp", bufs=3) as outp, \
             tc.tile_pool(name="psum", bufs=2, space="PSUM") as psum, \
             tc.tile_pool(name="dram", bufs=1, space="DRAM") as dram:

            # ---------------- constants / weights ----------------
            ident = const.tile([P, P], bf16)
            make_identity(nc, ident)
            identf = const.tile([P, P], f32)
            make_identity(nc, identf)

            x_nat = const.tile([P, RT, D2], f32)       # [r_in_tile, rt, d]
            nc.sync.dma_start(x_nat[:], x_d.rearrange("(t p) d -> p t d", p=P))

            w1_nat = const.tile([P, FC, D2], f32)      # W_att1 [j, d] tiles
            nc.sync.dma_start(w1_nat[:], w1_d.rearrange("(t p) d -> p t d", p=P))
            w2_nat = const.tile([P, D1], f32)          # W_att2 [i, j]
            nc.sync.dma_start(w2_nat[:], w2_d)
            wg_sb = const.tile([P, D1], f32r)          # W_gcn [d, f] natural
            nc.sync.dma_start(wg_sb[:], wg_d.bitcast(f32r))

            gamma_pf = const.tile([P, FC], f32)        # gamma[fc*128 + p]
            nc.sync.dma_start(gamma_pf[:], gamma_d.rearrange("(o p) -> p o", p=P))
            beta_pf = const.tile([P, FC], f32)
            nc.sync.dma_start(beta_pf[:], beta_d.rearrange("(o p) -> p o", p=P))

            # Pre-cast adj to fp16 into a block-tiled DRAM layout
            # [rs, cb, 512, 512] so mm1 can hardware-DMA-transpose each
            # 512x512 block straight from DRAM at full rate.
            adj16_d = dram.tile([RS, CB, RSW, CBW], bf16)
            for rs_ in range(RS):
                for rt in range(4):
                    for ch in range(2):           # 2 x 4096-col chunks
                        strip = precast.tile([P, 8, RSW], bf16, name=f"pc{rs_}_{rt}_{ch}", tag="pcstrip")
                        nc.gpsimd.dma_start(      # SWDGE cast f32->fp16
                            strip[:],
                            adj_d[rs_ * RSW + rt * P:rs_ * RSW + (rt + 1) * P,
                                  ch * 8 * CBW:(ch + 1) * 8 * CBW]
                            .rearrange("p (cb c) -> p cb c", c=CBW))
                        nc.sync.dma_start(
                            adj16_d[rs_, ch * 8:(ch + 1) * 8,
                                    rt * P:(rt + 1) * P, :]
                            .rearrange("cb p c -> p cb c"),
                            strip[:])

            # transpose attention weights (one-time)
            w1T = const.tile([P, FC, P], f32r)         # [d, jc, j]
            w2T = const.tile([P, FC, P], f32r)         # [j, jc, i]
            for t in range(FC):
                pt = psum.tile([P, 2 * P], f32, tag="ps_t", bufs=2)
                nc.tensor.transpose(pt[:, :P], w1_nat[:, t, :], identf[:])
                nc.tensor.transpose(pt[:, P:], w2_nat[:, t * P:(t + 1) * P],
                                    identf[:])
                nc.vector.tensor_copy(w1T[:, t, :], pt[:, :P])
                nc.vector.tensor_copy(w2T[:, t, :], pt[:, P:])

            # ---------------- stage A: attention + support ----------------
            # xT [d, r] via PE transposes (cast to bf16 on evacuation)
            xT = sa.tile([P, R], f32r)
            for g in range(2):
                pt = psum.tile([P, 4 * P], f32, tag="ps_t", bufs=2)
                for t in range(4):
                    nc.tensor.transpose(pt[:, t * P:(t + 1) * P],
                                        x_nat[:, g * 4 + t, :], identf[:])
                nc.vector.tensor_copy(xT[:, g * 4 * P:(g + 1) * 4 * P], pt[:])

            # h_attT[jc] = relu(W_att1 @ xT)   [j, r]
            hattT = sa.tile([P, FC, R], f32r)
            for jc in range(FC):
                for hh in range(2):
                    ph = psum.tile([P, RSW], f32, tag="ps_h", bufs=4)
                    nc.tensor.matmul(ph[:], w1T[:, jc, :],
                                     xT[:, hh * RSW:(hh + 1) * RSW],
                                     start=True, stop=True)
                    nc.vector.tensor_relu(hattT[:, jc, hh * RSW:(hh + 1) * RSW],
                                          ph[:])

            # per row-tile: score logits, softmax, cond, support
            sup_own = sa.tile([P, RT, D1], bf16)
            for rb in range(RT):
                ps_sc = psum.tile([P, P], f32, tag="ps_s", bufs=2)
                for jc in range(FC):
                    nc.tensor.matmul(ps_sc[:], hattT[:, jc, rb * P:(rb + 1) * P],
                                     w2T[:, jc, :],
                                     start=(jc == 0), stop=(jc == FC - 1))
                mxn = work.tile([P, 1], f32, tag="mxn")
                nc.vector.reduce_max(out=mxn[:], in_=ps_sc[:], axis=AX.X,
                                     negate=True)
                esc = work.tile([P, P], f32, tag="esc")
                ssum = work.tile([P, 1], f32, tag="ssum")
                nc.scalar.activation(esc[:], ps_sc[:], ACT.Exp,
                                     bias=mxn[:], scale=1.0, accum_out=ssum[:])
                rinv = work.tile([P, 1], f32, tag="rinv")
                nc.vector.reciprocal(rinv[:], ssum[:])
                # cond = (esc * rinv) * x
                cond = work.tile([P, P], f32, tag="cond")
                nc.vector.scalar_tensor_tensor(cond[:], esc[:], rinv[:],
                                               x_nat[:, rb, :],
                                               op0=ALU.mult, op1=ALU.mult)
                pt = psum.tile([P, P], f32, tag="ps_t", bufs=2)
                nc.tensor.transpose(pt[:], cond[:], identf[:])
                condT = work.tile([P, P], f32r, tag="condT")
                nc.vector.tensor_copy(condT[:], pt[:])
                ps_sup = psum.tile([P, D1], f32, tag="ps_s", bufs=2)
                nc.tensor.matmul(ps_sup[:], condT[:], wg_sb[:],
                                 start=True, stop=True)
                nc.vector.tensor_copy(sup_own[:, rb, :], ps_sup[:])

            # AllGather support (bf16) -> full [N, D1]
            ag_sup_in = dram.tile([R, D1], bf16)
            ag_sup_out = dram.tile([N, D1], bf16, addr_space="Shared")
            nc.sync.dma_start(ag_sup_in[:].rearrange("(t p) f -> p t f", p=P),
                              sup_own[:])
            nc.gpsimd.collective_compute(
                "AllGather", ALU.bypass, replica_groups=rg,
                ins=[ag_sup_in[:].opt()], outs=[ag_sup_out[:].opt()])

            # full support cached in SBUF (bf16, 64 KB/partition)
            sup_sb = sa.tile([P, CB * 4, D1], bf16)
            for cb in range(CB):
                nc.sync.dma_start(
                    sup_sb[:, cb * 4:(cb + 1) * 4, :],
                    ag_sup_out[cb * CBW:(cb + 1) * CBW, :]
                    .rearrange("(t p) f -> p t f", p=P))

            # ---------------- mm1: hT = relu(adj @ support)^T ----------------
            # BN stats folded into the per-(rs,fc) epilogue:
            #   pstats[:, 0:8]  partial sums   (rs, fc)
            #   pstats[:, 8:16] partial sumsq  (rs, fc)
            hT = sa.tile([P, FC, R], bf16)
            pstats = sa.tile([P, 4 * FC], f32)
            sq_scr = sa.tile([P, RSW], bf16)
            for rs in range(RS):
                phs = [psum.tile([P, RSW], f32, tag="ps_h", bufs=4,
                                 name=f"ps_h{rs}_{i}") for i in range(FC)]
                for cb in range(CB):
                    adjT = adjtp.tile([P, 4, RSW], bf16)
                    nc.sync.dma_start_transpose(adjT[:], adj16_d[rs, cb])
                    for cc in range(4):
                        for fc in range(FC):
                            nc.tensor.matmul(
                                phs[fc][:],
                                sup_sb[:, cb * 4 + cc, fc * P:(fc + 1) * P],
                                adjT[:, cc, :],
                                start=(cb == 0 and cc == 0),
                                stop=(cb == CB - 1 and cc == 3))
                for fc in range(FC):
                    sl = hT[:, fc, rs * RSW:(rs + 1) * RSW]
                    nc.vector.tensor_relu(sl, phs[fc][:])
                    nc.vector.reduce_sum(
                        out=pstats[:, rs * FC + fc:rs * FC + fc + 1],
                        in_=sl, axis=AX.X)
                    nc.scalar.activation(
                        sq_scr[:], sl, ACT.Square,
                        accum_out=pstats[:, 2 * FC + rs * FC + fc:
                                         2 * FC + rs * FC + fc + 1])

            # combine rs partials -> stats [sum(4) | sumsq(4)]
            stats = sa.tile([P, 2 * FC], f32)
            nc.vector.tensor_add(stats[:, :FC], pstats[:, :FC],
                                 pstats[:, FC:2 * FC])
            nc.vector.tensor_add(stats[:, FC:], pstats[:, 2 * FC:3 * FC],
                                 pstats[:, 3 * FC:])

            # ---------------- BatchNorm stats AllReduce ----------------
            bn_in = dram.tile([P, 2 * FC], f32)
            bn_out = dram.tile([P, 2 * FC], f32, addr_space="Shared")
            nc.sync.dma_start(bn_in[:], stats[:])
            nc.gpsimd.collective_compute(
                "AllReduce", ALU.add, replica_groups=rg,
                ins=[bn_in[:].opt()], outs=[bn_out[:].opt()])
            gstats = sa.tile([P, 2 * FC], f32)
            nc.sync.dma_start(gstats[:], bn_out[:])

            mean = sa.tile([P, FC], f32)
            var = sa.tile([P, FC], f32)
            sd = sa.tile([P, FC], f32)
            isd = sa.tile([P, FC], f32)
            scl = sa.tile([P, FC], f32)
            bia = sa.tile([P, FC], f32)
            nc.vector.tensor_scalar_mul(mean[:], gstats[:, :FC], 1.0 / N)
            nc.vector.tensor_scalar_mul(var[:], gstats[:, FC:], 1.0 / N)
            nc.vector.tensor_tensor(sd[:], mean[:], mean[:], ALU.mult)
            nc.vector.tensor_sub(var[:], var[:], sd[:])
            nc.vector.tensor_scalar_add(var[:], var[:], BN_EPS)
            nc.scalar.activation(sd[:], var[:], ACT.Sqrt)
            nc.vector.reciprocal(isd[:], sd[:])
            nc.vector.tensor_tensor(scl[:], gamma_pf[:], isd[:], ALU.mult)
            nc.vector.tensor_tensor(bia[:], mean[:], scl[:], ALU.mult)
            nc.vector.tensor_sub(bia[:], beta_pf[:], bia[:])

            # z^T = hT * scale + bias   (per-partition scalars), bf16
            zT = sa.tile([P, FC, R], bf16)
            for fc in range(FC):
                nc.vector.tensor_scalar(zT[:, fc, :], hT[:, fc, :],
                                        scl[:, fc:fc + 1], bia[:, fc:fc + 1],
                                        op0=ALU.mult, op1=ALU.add)

            # AllGather z^T (fp16)
            ag_z_in = dram.tile([D1, R], bf16)
            ag_z_out = dram.tile([NCORES * D1, R], bf16, addr_space="Shared")
            nc.sync.dma_start(
                ag_z_in[:].rearrange("(t p) r -> p t r", p=P), zT[:])
            nc.gpsimd.collective_compute(
                "AllGather", ALU.bypass, replica_groups=rg,
                ins=[ag_z_in[:].opt()], outs=[ag_z_out[:].opt()])

            # ---------------- mm2: out_blk = z_blk @ z^T ----------------
            # full-width rhs slab per source block j; each zT weight tile
            # feeds two N=512 matmuls (both column halves) so LDWEIGHTS
            # amortizes across 2 matmuls.
            for j in range(NCORES):
                rhs_slab = rhsp.tile([P, FC, R], bf16)
                nc.sync.dma_start(
                    rhs_slab[:],
                    ag_z_out[j * D1:(j + 1) * D1, :]
                    .rearrange("(t p) n -> p t n", p=P))
                for rb in range(RT):
                    out_sb = outp.tile([P, 2, D1], f32)
                    ps_a = psum.tile([P, D1], f32, tag="ps_h", bufs=4)
                    ps_b = psum.tile([P, D1], f32, tag="ps_h", bufs=4)
                    for fc in range(FC):
                        w = zT[:, fc, rb * P:(rb + 1) * P]
                        nc.tensor.matmul(ps_a[:], w, rhs_slab[:, fc, :D1],
                                         start=(fc == 0), stop=(fc == FC - 1))
                        nc.tensor.matmul(ps_b[:], w, rhs_slab[:, fc, D1:],
                                         start=(fc == 0), stop=(fc == FC - 1))
                    nc.vector.tensor_copy(out_sb[:, 0, :], ps_a[:])
                    nc.scalar.copy(out_sb[:, 1, :], ps_b[:])
                    nc.sync.dma_start(
                        out_d[rb * P:(rb + 1) * P, j * R:(j + 1) * R]
                        .rearrange("n (t d) -> n t d", t=2),
                        out_sb[:])

    nc.compile()
    return nc


_NC_CACHE = None


def _get_nc():
    global _NC_CACHE
    if _NC_CACHE is None:
        _NC_CACHE = build_kernel()
    return _NC_CACHE


def kernel(encoder_layer_2, adj, W_att1, W_att2, W_gcn, gamma, beta,
           _trace=False):
    nc = _get_nc()
    x = np.ascontiguousarray(encoder_layer_2, dtype=np.float32)
    adj = np.ascontiguousarray(adj, dtype=np.float32)
    shared = {
        "W_att1": np.ascontiguousarray(W_att1, dtype=np.float32),
        "W_att2": np.ascontiguousarray(W_att2, dtype=np.float32),
        "W_gcn": np.ascontiguousarray(W_gcn, dtype=np.float32),
        "gamma": np.ascontiguousarray(gamma, dtype=np.float32),
        "beta": np.ascontiguousarray(beta, dtype=np.float32),
    }
    in_maps = []
    for i in range(NCORES):
        m = dict(shared)
        m["x_blk"] = np.ascontiguousarray(x[i * R:(i + 1) * R])
        m["adj_blk"] = np.ascontiguousarray(adj[i * R:(i + 1) * R])
        in_maps.append(m)
    res = bass_utils.run_bass_kernel_spmd(
        nc, in_maps, core_ids=list(range(NCORES)), trace=_trace)
    out = np.concatenate([res.results[i]["out_blk"] for i in range(NCORES)],
                         axis=0)
    if _trace:
        return out, res
    return out


# revision 17
# speedup vs baseline: 1.1776x; 1.1776x over previous
"""Trainium2 Bass kernel for nn_GCNModelFeaAttScatStructureOnlyVAE.

Computation (reference):
    h_att  = relu(x @ W_att1.T)                    [N, D1]
    score  = softmax(h_att @ W_att2.T, axis=1)     [N, D2]
    cond   = x * score                             [N, D2]
    support= cond @ W_gcn                          [N, D1]
    h      = relu(adj @ support)                   [N, D1]
    z      = batchnorm(h) (batch stats, biased var) * gamma + beta
    out    = z @ z.T                               [N, N]

Distribution (8 NeuronCores, row-sharded nodes):
  Each core owns R = N/8 rows of x/adj/out. Stage A (attention+support) is
  computed per-core on its row block; support is AllGathered (bf16). The
  big matmul adj_blk @ support runs in TRANSPOSED orientation
  (h^T[f, r] = sum_c support[c, f] * adjT[c, r]) so BatchNorm becomes a
  free-dim reduction and z^T comes out pre-transposed for the z @ z.T
  decode. adj tiles are cast to bf16 during the DMA load (SWDGE) and
  transposed on-chip by the PE. BN stats are reduced inside the matmul-1
  epilogue and AllReduced (4 KB); z^T is AllGathered in two node-halves
  (bf16) so the second half's transfer overlaps the first half's matmuls.

  Matmul operands are bf16 (full-rate PE + fast weight load); all
  accumulation is fp32 in PSUM; softmax/BN statistics are fp32.
"""
import os
import sys

for _p in ("/opt/trn_rl_repo", "/root/.axon_site/_ro/trn_rl_repo"):
    if _p not in sys.path:
        sys.path.append(_p)

import numpy as np

import concourse.bass as bass
import concourse.mybir as mybir
import concourse.tile as tile
from concourse import bacc, bass_utils
from concourse.masks import make_identity

f32 = mybir.dt.float32
f32r = mybir.dt.float32r
bf16 = mybir.dt.float16  # fp16: same PE rate as bf16, 4x finer mantissa
AX = mybir.AxisListType
ALU = mybir.AluOpType
ACT = mybir.ActivationFunctionType

N = 8192
D2 = 128
D1 = 512
NCORES = 8
R = N // NCORES          # 1024 rows per core
BN_EPS = 1e-5
P = 128

RT = R // P              # 8 row tiles per core
FC = D1 // P             # 4 feature chunks
RS = 2                   # row-slices per core in mm1 (512 rows each)
RSW = R // RS            # 512
CB = 16                  # c-superchunks (512 cols each)
CBW = N // CB            # 512


def build_kernel():
    nc = bacc.Bacc("TRN2", target_bir_lowering=False, debug=False,
                   num_devices=NCORES)

    x_d = nc.dram_tensor("x_blk", [R, D2], f32, kind="ExternalInput").ap()
    adj_d = nc.dram_tensor("adj_blk", [R, N], f32, kind="ExternalInput").ap()
    w1_d = nc.dram_tensor("W_att1", [D1, D2], f32, kind="ExternalInput").ap()
    w2_d = nc.dram_tensor("W_att2", [D2, D1], f32, kind="ExternalInput").ap()
    wg_d = nc.dram_tensor("W_gcn", [D2, D1], f32, kind="ExternalInput").ap()
    gamma_d = nc.dram_tensor("gamma", [D1], f32, kind="ExternalInput").ap()
    beta_d = nc.dram_tensor("beta", [D1], f32, kind="ExternalInput").ap()
    out_d = nc.dram_tensor("out_blk", [R, N], f32, kind="ExternalOutput").ap()

    rg = [list(range(NCORES))]

    with tile.TileContext(nc) as tc:
        with tc.tile_pool(name="const", bufs=1) as const, \
             tc.tile_pool(name="stagea", bufs=1) as sa, \
             tc.tile_pool(name="work", bufs=3) as work, \
             tc.tile_pool(name="adjp", bufs=3) as adjp, \
             tc.tile_pool(name="adjtp", bufs=6) as adjtp, \
             tc.tile_pool(name="rhsp", bufs=3) as rhsp, \
             tc.tile_pool(name="outp", bufs=4) as outp, \
             tc.tile_pool(name="psum", bufs=2, space="PSUM") as psum, \
             tc.tile_pool(name="dram", bufs=1, space="DRAM") as dram:

            # ---------------- constants / weights ----------------
            ident = const.tile([P, P], bf16)
            make_identity(nc, ident)
            identf = const.tile([P, P], f32)
            make_identity(nc, identf)

            x_nat = const.tile([P, RT, D2], f32)       # [r_in_tile, rt, d]
            nc.sync.dma_start(x_nat[:], x_d.rearrange("(t p) d -> p t d", p=P))

            w1_nat = const.tile([P, FC, D2], f32)      # W_att1 [j, d] tiles
            nc.sync.dma_start(w1_nat[:], w1_d.rearrange("(t p) d -> p t d", p=P))
            w2_nat = const.tile([P, D1], f32)          # W_att2 [i, j]
            nc.sync.dma_start(w2_nat[:], w2_d)
            wg_sb = const.tile([P, D1], bf16)          # W_gcn [d, f] natural
            nc.gpsimd.dma_start(wg_sb[:], wg_d)        # SWDGE cast f32->fp16

            gamma_pf = const.tile([P, FC], f32)        # gamma[fc*128 + p]
            nc.sync.dma_start(gamma_pf[:], gamma_d.rearrange("(o p) -> p o", p=P))
            beta_pf = const.tile([P, FC], f32)
            nc.sync.dma_start(beta_pf[:], beta_d.rearrange("(o p) -> p o", p=P))

            # transpose attention weights (one-time)
            w1T = const.tile([P, FC, P], bf16)         # [d, jc, j]
            w2T = const.tile([P, FC, P], bf16)         # [j, jc, i]
            for t in range(FC):
                pt = psum.tile([P, 2 * P], f32, tag="ps_t", bufs=2)
                nc.tensor.transpose(pt[:, :P], w1_nat[:, t, :], identf[:])
                nc.tensor.transpose(pt[:, P:], w2_nat[:, t * P:(t + 1) * P],
                                    identf[:])
                nc.vector.tensor_copy(w1T[:, t, :], pt[:, :P])
                nc.vector.tensor_copy(w2T[:, t, :], pt[:, P:])

            # ---------------- stage A: attention + support ----------------
            # xT [d, r] via PE transposes (cast to bf16 on evacuation)
            xT = sa.tile([P, R], bf16)
            for g in range(2):
                pt = psum.tile([P, 4 * P], f32, tag="ps_t", bufs=2)
                for t in range(4):
                    nc.tensor.transpose(pt[:, t * P:(t + 1) * P],
                                        x_nat[:, g * 4 + t, :], identf[:])
                nc.vector.tensor_copy(xT[:, g * 4 * P:(g + 1) * 4 * P], pt[:])

            # h_attT[jc] = relu(W_att1 @ xT)   [j, r]
            hattT = sa.tile([P, FC, R], bf16)
            for jc in range(FC):
                for hh in range(2):
                    ph = psum.tile([P, RSW], f32, tag="ps_h", bufs=4)
                    nc.tensor.matmul(ph[:], w1T[:, jc, :],
                                     xT[:, hh * RSW:(hh + 1) * RSW],
                                     start=True, stop=True)
                    nc.vector.tensor_relu(hattT[:, jc, hh * RSW:(hh + 1) * RSW],
                                          ph[:])

            # per row-tile: score logits, softmax, cond, support
            sup_own = sa.tile([P, RT, D1], bf16)
            for rb in range(RT):
                ps_sc = psum.tile([P, P], f32, tag="ps_s", bufs=2)
                for jc in range(FC):
                    nc.tensor.matmul(ps_sc[:], hattT[:, jc, rb * P:(rb + 1) * P],
                                     w2T[:, jc, :],
                                     start=(jc == 0), stop=(jc == FC - 1))
                mxn = work.tile([P, 1], f32, tag="mxn")
                nc.vector.reduce_max(out=mxn[:], in_=ps_sc[:], axis=AX.X,
                                     negate=True)
                esc = work.tile([P, P], f32, tag="esc")
                ssum = work.tile([P, 1], f32, tag="ssum")
                nc.scalar.activation(esc[:], ps_sc[:], ACT.Exp,
                                     bias=mxn[:], scale=1.0, accum_out=ssum[:])
                rinv = work.tile([P, 1], f32, tag="rinv")
                nc.vector.reciprocal(rinv[:], ssum[:])
                # cond = (esc * rinv) * x
                cond = work.tile([P, P], f32, tag="cond")
                nc.vector.scalar_tensor_tensor(cond[:], esc[:], rinv[:],
                                               x_nat[:, rb, :],
                                               op0=ALU.mult, op1=ALU.mult)
                pt = psum.tile([P, P], f32, tag="ps_t", bufs=2)
                nc.tensor.transpose(pt[:], cond[:], identf[:])
                condT = work.tile([P, P], bf16, tag="condT")
                nc.vector.tensor_copy(condT[:], pt[:])
                ps_sup = psum.tile([P, D1], f32, tag="ps_s", bufs=2)
                nc.tensor.matmul(ps_sup[:], condT[:], wg_sb[:],
                                 start=True, stop=True)
                nc.vector.tensor_copy(sup_own[:, rb, :], ps_sup[:])

            # AllGather support (bf16) -> full [N, D1]
            ag_sup_in = dram.tile([R, D1], bf16)
            ag_sup_out = dram.tile([N, D1], bf16, addr_space="Shared")
            nc.sync.dma_start(ag_sup_in[:].rearrange("(t p) f -> p t f", p=P),
                              sup_own[:])
            nc.gpsimd.collective_compute(
                "AllGather", ALU.bypass, replica_groups=rg,
                ins=[ag_sup_in[:].opt()], outs=[ag_sup_out[:].opt()])

            # full support cached in SBUF (bf16, 64 KB/partition)
            sup_sb = sa.tile([P, CB * 4, D1], bf16)
            for cb in range(CB):
                nc.sync.dma_start(
                    sup_sb[:, cb * 4:(cb + 1) * 4, :],
                    ag_sup_out[cb * CBW:(cb + 1) * CBW, :]
                    .rearrange("(t p) f -> p t f", p=P))

            # ---------------- mm1: hT = relu(adj @ support)^T ----------------
            # BN stats folded into the per-(rs,fc) epilogue:
            #   pstats[:, 0:8]  partial sums   (rs, fc)
            #   pstats[:, 8:16] partial sumsq  (rs, fc)
            hT = sa.tile([P, FC, R], bf16)
            pstats = sa.tile([P, 4 * FC], f32)
            sq_scr = sa.tile([P, RSW], bf16)
            for rs in range(RS):
                phs = [psum.tile([P, RSW], f32, tag="ps_h", bufs=4,
                                 name=f"ps_h{rs}_{i}") for i in range(FC)]
                for cb in range(CB):
                    adj_nat = adjp.tile([P, 4, CBW], bf16)
                    nc.gpsimd.dma_start(          # SWDGE cast f32->fp16
                        adj_nat[:],
                        adj_d[rs * RSW:(rs + 1) * RSW,
                              cb * CBW:(cb + 1) * CBW]
                        .rearrange("(t p) c -> p t c", p=P))
                    adjT = adjtp.tile([P, 4, RSW], bf16)
                    for cc in range(4):
                        pt = psum.tile([P, RSW], bf16, tag="ps_t", bufs=2)
                        for rt in range(4):
                            nc.tensor.transpose(
                                pt[:, rt * P:(rt + 1) * P],
                                adj_nat[:, rt, cc * P:(cc + 1) * P], ident[:])
                        nc.vector.tensor_copy(adjT[:, cc, :], pt[:])
                        for fc in range(FC):
                            nc.tensor.matmul(
                                phs[fc][:],
                                sup_sb[:, cb * 4 + cc, fc * P:(fc + 1) * P],
                                adjT[:, cc, :],
                                start=(cb == 0 and cc == 0),
                                stop=(cb == CB - 1 and cc == 3))
                for fc in range(FC):
                    sl = hT[:, fc, rs * RSW:(rs + 1) * RSW]
                    nc.vector.tensor_relu(sl, phs[fc][:])
                    nc.vector.reduce_sum(
                        out=pstats[:, rs * FC + fc:rs * FC + fc + 1],
                        in_=sl, axis=AX.X)
                    nc.scalar.activation(
                        sq_scr[:], sl, ACT.Square,
                        accum_out=pstats[:, 2 * FC + rs * FC + fc:
                                         2 * FC + rs * FC + fc + 1])

            # combine rs partials -> stats [sum(4) | sumsq(4)]
            stats = sa.tile([P, 2 * FC], f32)
            nc.vector.tensor_add(stats[:, :FC], pstats[:, :FC],
                                 pstats[:, FC:2 * FC])
            nc.vector.tensor_add(stats[:, FC:], pstats[:, 2 * FC:3 * FC],
                                 pstats[:, 3 * FC:])

            # ---------------- BatchNorm stats AllReduce ----------------
            bn_in = dram.tile([P, 2 * FC], f32)
            bn_out = dram.tile([P, 2 * FC], f32, addr_space="Shared")
            nc.sync.dma_start(bn_in[:], stats[:])
            nc.gpsimd.collective_compute(
                "AllReduce", ALU.add, replica_groups=rg,
                ins=[bn_in[:].opt()], outs=[bn_out[:].opt()])
            gstats = sa.tile([P, 2 * FC], f32)
            nc.sync.dma_start(gstats[:], bn_out[:])

            mean = sa.tile([P, FC], f32)
            var = sa.tile([P, FC], f32)
            sd = sa.tile([P, FC], f32)
            isd = sa.tile([P, FC], f32)
            scl = sa.tile([P, FC], f32)
            bia = sa.tile([P, FC], f32)
            nc.vector.tensor_scalar_mul(mean[:], gstats[:, :FC], 1.0 / N)
            nc.vector.tensor_scalar_mul(var[:], gstats[:, FC:], 1.0 / N)
            nc.vector.tensor_tensor(sd[:], mean[:], mean[:], ALU.mult)
            nc.vector.tensor_sub(var[:], var[:], sd[:])
            nc.vector.tensor_scalar_add(var[:], var[:], BN_EPS)
            nc.scalar.activation(sd[:], var[:], ACT.Sqrt)
            nc.vector.reciprocal(isd[:], sd[:])
            nc.vector.tensor_tensor(scl[:], gamma_pf[:], isd[:], ALU.mult)
            nc.vector.tensor_tensor(bia[:], mean[:], scl[:], ALU.mult)
            nc.vector.tensor_sub(bia[:], beta_pf[:], bia[:])

            # z^T = hT * scale + bias   (per-partition scalars), bf16
            zT = sa.tile([P, FC, R], bf16)
            for fc in range(FC):
                nc.vector.tensor_scalar(zT[:, fc, :], hT[:, fc, :],
                                        scl[:, fc:fc + 1], bia[:, fc:fc + 1],
                                        op0=ALU.mult, op1=ALU.add)

            # AllGather z^T (fp16)
            ag_z_in = dram.tile([D1, R], bf16)
            ag_z_out = dram.tile([NCORES * D1, R], bf16, addr_space="Shared")
            nc.sync.dma_start(
                ag_z_in[:].rearrange("(t p) r -> p t r", p=P), zT[:])
            nc.gpsimd.collective_compute(
                "AllGather", ALU.bypass, replica_groups=rg,
                ins=[ag_z_in[:].opt()], outs=[ag_z_out[:].opt()])

            # ---------------- mm2: out_blk = z_blk @ z^T ----------------
            # full-width rhs slab per source block j; each zT weight tile
            # feeds two N=512 matmuls (both column halves) so LDWEIGHTS
            # amortizes across 2 matmuls.
            for j in range(NCORES):
                rhs_slab = rhsp.tile([P, FC, R], bf16)
                nc.sync.dma_start(
                    rhs_slab[:],
                    ag_z_out[j * D1:(j + 1) * D1, :]
                    .rearrange("(t p) n -> p t n", p=P))
                for rb in range(RT):
                    out_sb = outp.tile([P, 2, D1], f32)
                    ps_a = psum.tile([P, D1], f32, tag="ps_h", bufs=4)
                    ps_b = psum.tile([P, D1], f32, tag="ps_h", bufs=4)
                    for fc in range(FC):
                        w = zT[:, fc, rb * P:(rb + 1) * P]
                        nc.tensor.matmul(ps_a[:], w, rhs_slab[:, fc, :D1],
                                         start=(fc == 0), stop=(fc == FC - 1))
                        nc.tensor.matmul(ps_b[:], w, rhs_slab[:, fc, D1:],
                                         start=(fc == 0), stop=(fc == FC - 1))
                    nc.vector.tensor_copy(out_sb[:, 0, :], ps_a[:])
                    nc.scalar.copy(out_sb[:, 1, :], ps_b[:])
                    nc.sync.dma_start(
                        out_d[rb * P:(rb + 1) * P, j * R:(j + 1) * R]
                        .rearrange("n (t d) -> n t d", t=2),
                        out_sb[:])

    nc.compile()
    return nc


_NC_CACHE = None


def _get_nc():
    global _NC_CACHE
    if _NC_CACHE is None:
        _NC_CACHE = build_kernel()
    return _NC_CACHE


def kernel(encoder_layer_2, adj, W_att1, W_att2, W_gcn, gamma, beta,
           _trace=False):
    nc = _get_nc()
    x = np.ascontiguousarray(encoder_layer_2, dtype=np.float32)
    adj = np.ascontiguousarray(adj, dtype=np.float32)
    shared = {
        "W_att1": np.ascontiguousarray(W_att1, dtype=np.float32),
        "W_att2": np.ascontiguousarray(W_att2, dtype=np.float32),
        "W_gcn": np.ascontiguousarray(W_gcn, dtype=np.float32),
        "gamma": np.ascontiguousarray(gamma, dtype=np.float32),
        "beta": np.ascontiguousarray(beta, dtype=np.float32),
    }
    in_maps = []
    for i in range(NCORES):
        m = dict(shared)
        m["x_blk"] = np.ascontiguousarray(x[i * R:(i + 1) * R])
        m["adj_blk"] = np.ascontiguousarray(adj[i * R:(i + 1) * R])
        in_maps.append(m)
    res = bass_utils.run_bass_kernel_spmd(
        nc, in_maps, core_ids=list(range(NCORES)), trace=_trace)
    out = np.concatenate([res.results[i]["out_blk"] for i in range(NCORES)],
                         axis=0)
    if _trace:
        return out, res
    return out
